# revision 30
# baseline (speedup 1.0000x reference)
"""Trainium2 Bass kernel for the additive coupling flow (nn_Additive_flow).

Math: 65 sequential steps. Step s (i = idx[s]) updates column i of z:
    z[:, i] += MLP_s(z with cols i<->63 swapped, first 63 cols) + b3[s]
Reformulated with no data permutation:
    h1 = relu(z @ W1e[s])      W1e[s] = [W1[s]; 0] with rows i,63 swapped
    h2 = relu(h1 @ W2[s])
    z[:, i] += h2 @ w3[s]      (plus biases; zero in practice)
Finally out = exp(s_vec) * z.

Device layout ("split-sample"): each core's 16384 samples are split in two
halves of 8192; SBUF state z is [128, 8192] bf16 where partitions 0:63
hold features of half A and 64:127 of half B.

v4+ mixed precision (fp8 DoubleRow): a DR matmul contracts K=256 (two
128-row subtiles) in the same wall-time as one K=128 bf16 matmul
(measured 221.6 ns per 512-col mm either way) = 2x MAC rate. Per-step
schedule from a numpy error study (sim transfers to HW within ~2%):
L2 runs fp8-DR except the 4 most error-sensitive steps {26,27,50,51}
(which alone carry ~50% of the full-fp8 variance); L3 runs fp8-DR on 41
cheap steps; L1 stays bf16 (z would need an extra fp8 cast + error).
Weight-side hi/lo fp8 tricks are pointless on real HW (2-term = bf16
cost), activation-side hi/lo loses to the elementwise wall.
  - L1 (bf16): block-diagonal weight tiles [W1e_c;0]/[0;W1e_c], 4
    MMs/macro into two 2-bank PSUM tiles (h1a=half A [k0|k1], h1b=B).
  - L2 fp8: 4 DR MMs/macro (A-mc0, A-mc1, B-mc0, B-mc1->pD bank);
    bf16 fallback: 8 MMs.
  - L3 fp8: 2 DR MMs accumulating into the pD bank (reused after the
    B-mc1 evacuation); bf16: 4 MMs with M=128 zero-padded scatter tiles.
  - relu+dtype-convert is free in the evacuation op: ACT does h1a
    [128,1024], h2A [128,1024], h2B-mc1 [128,512] (2.69 us/macro); DVE
    does h1b, h2B-mc0, zadd (2.51 us/macro). The PSUM->SBUF elementwise
    wall (4608 el/partition/macro; Pool engine cannot access PSUM) is the
    binding constraint together with ACT-PE dependency alignment.
Emission per macro: [L1-mms(m)] [stageB-front(m-1)] [h1-relus(m)]
[zadd(m-1)]. PSUM: h1 4 banks + h2A 2 + h2B 1 + pD 1 = 8 exactly.
Weights stream per step (bf16 L1 tensor + bf16/fp8 L2L3 tensors, only
the needed slices DMA'd) with 2-step prefetch. exp(s) is precomputed on
host (device Exp table costs ~1e-3 noise); output is stored/DMA'd in
bf16 (halves output traffic, +0.01e-2 err). Measured: 3.134 ms at rel
err 1.61e-2 (budget 2e-2) vs 3.707 ms bf16 baseline. Per-macro steady
state: ACT busy 2697 ns (binding on L3-fp8 steps, +~140 ns semaphore
tail), PE 2678 avg (binding on L3-bf16 steps), DVE 2454. Measured dead
ends: walrus --enable-ldw-opt=true crashes codegen (would reclaim
~250us of LDWEIGHTS); splitting one PSUM region's evacuation across
ACT+DVE serializes (-35%); monolithic 4-bank h1 psum tile serializes
on the whole-tile WAR (-28%). Note: the device occasionally lands in a
transient ~+0.6ms slow mode on a fresh process; rerunning recovers.
"""

import os
import sys

for _p in ("/opt/trn_rl_repo", "/root/.axon_site/_ro/trn_rl_repo"):
    if os.path.isdir(_p) and _p not in sys.path:
        sys.path.append(_p)

import numpy as np
import concourse.bass as bass
import concourse.bacc as bacc
import concourse.mybir as mybir
from concourse.tile import TileContext
from concourse.bass_utils import run_bass_kernel_spmd

NCORES = 8
B = 131072
N = 64          # latent dim
S = 65          # coupling steps
H = 256         # MLP width
BSH = B // NCORES      # 16384 samples per core
HALF = BSH // 2        # 8192 samples per partition-half
FD = 512               # matmul moving free-dim; one macro = 2*FD samples
NMACRO = HALF // FD    # 16
WCOLS = 1536           # per-step packed weight columns

F32 = mybir.dt.float32
F32R = mybir.dt.float32r
BF16 = mybir.dt.bfloat16
AF = mybir.ActivationFunctionType
ALU = mybir.AluOpType

LAST_RESULT = None  # test.py reads exec_time_ns from here

_HOOK_SRC = """\
_hook = None


def set_axon_ntff_profile_hook(h):
    global _hook
    _hook = h


def get_axon_ntff_profile_hook():
    return _hook
"""


def _ensure_ntff_hook():
    """Install the axon NTFF profiling hook if the image's antenv lacks
    axon_hooks (degrades silently; tracing is optional)."""
    try:
        from antenv.axon_hooks import get_axon_ntff_profile_hook

        if get_axon_ntff_profile_hook() is not None:
            return
        have_module = True
    except ImportError:
        have_module = False
    try:
        import antenv

        if not have_module:
            ext = "/tmp/axon_hooks_ext"
            os.makedirs(ext, exist_ok=True)
            p = os.path.join(ext, "axon_hooks.py")
            if not os.path.exists(p):
                with open(p, "w") as f:
                    f.write(_HOOK_SRC)
            if ext not in antenv.__path__:
                antenv.__path__.append(ext)
        from antenv.axon_hooks import (
            get_axon_ntff_profile_hook,
            set_axon_ntff_profile_hook,
        )

        if get_axon_ntff_profile_hook() is None:
            from trn_agent_boot.trn_boot import _ntff_profile_via_ctypes

            hook = _ntff_profile_via_ctypes("/opt/axon/libaxon_pjrt.so")
            if hook is not None:
                set_axon_ntff_profile_hook(hook)
    except Exception:
        pass


F8 = mybir.dt.float8e4
DRMODE = mybir.MatmulPerfMode.DoubleRow

# Per-step precision schedule (from numpy error study, budget 2e-2):
# L2 in fp8-DoubleRow except the 4 most error-sensitive steps (26,27,50,51
# carry ~50% of the full-fp8 variance); L3 bf16 (fp8-L3 buys <3% speed once
# the ACT/DVE elementwise wall binds, not worth the error). fp8-DR
# contracts K=256 per pass = 2x bf16 MAC rate on the PE (measured 221.6ns
# per 512-col matmul, same wall-time as a K=128 bf16 matmul).
L2_SKIP = {26, 27, 50, 51}
L3_ON = (set(range(0, 8)) | set(range(16, 24)) | set(range(40, 48))
         | set(range(56, 65)))
STEP_CFG = [(st not in L2_SKIP, st in L3_ON) for st in range(S)]


def build_program_v4(nsteps=S, nmacro=NMACRO, cfg=None):
    """v4: per-step mixed-precision. L1 always bf16 (block-diag tiles);
    L2/L3 per STEP_CFG either bf16 (v3 path) or fp8 DoubleRow (K=256 in
    one matmul). h1/h2 relu outputs are written directly in the dtype the
    next matmul needs (fp8 conversion is free in the ACT/DVE op).
    PSUM: h1a+h1b 4 banks, h2A 2 banks, h2B 1 bank, pD 1 bank; half-B's
    mc1 L2 output goes to the pD bank (sequentially reused by L3)."""
    if cfg is None:
        cfg = STEP_CFG
    half = nmacro * FD
    nc = bacc.Bacc("TRN2", target_bir_lowering=False, debug=False)

    xt = nc.dram_tensor("xt", [128, half], BF16, kind="ExternalInput")
    wb_d = nc.dram_tensor("wb", [128, nsteps * 512], BF16, kind="ExternalInput")
    w23b_d = nc.dram_tensor("w23b", [128, nsteps * 1024], BF16,
                            kind="ExternalInput")
    w23f_d = nc.dram_tensor("w23f", [128, nsteps * 1024], F8,
                            kind="ExternalInput")
    s_d = nc.dram_tensor("sv", [128, 1], F32, kind="ExternalInput")
    # bf16 output: halves the output DMA and enables the DVE 2x mode on
    # the exp(s) scale op; adds only ~0.2% per-element rounding on the
    # final values (1.604e-2 -> 1.606e-2 total, negligible vs 2e-2 gate)
    out_d = nc.dram_tensor("out", [128, half], BF16, kind="ExternalOutput")

    with TileContext(nc) as tc:
        with (
            tc.tile_pool(name="zpool", bufs=1) as zp,
            tc.tile_pool(name="consts", bufs=1) as cp,
            tc.tile_pool(name="w1pool", bufs=12) as w1p,
            tc.tile_pool(name="w23bpool", bufs=12) as w23bp,
            tc.tile_pool(name="w23fpool", bufs=12) as w23fp,
            tc.tile_pool(name="h1pool", bufs=4) as h1p,
            tc.tile_pool(name="h2pool", bufs=4) as h2p,
            tc.tile_pool(name="opool", bufs=3) as op,
            tc.tile_pool(name="psH1", bufs=1, space="PSUM") as pH1,
            tc.tile_pool(name="psH2", bufs=1, space="PSUM") as pH2,
            tc.tile_pool(name="psD", bufs=1, space="PSUM") as pD,
        ):
            # sv carries exp(s) precomputed on host (device Exp is a table
            # approximation; exact host value removes a ~1e-3 noise floor)
            exps = cp.tile([128, 1], F32, tag="exps")
            nc.sync.dma_start(exps[:], s_d[:])

            def fetch_weights(st):
                l2f8, l3f8 = cfg[st]
                wt1 = w1p.tile([128, 512], BF16, tag="w1")
                nc.sync.dma_start(wt1[:], wb_d[:, bass.ts(st, 512)])
                wtb = wtf = None
                if l2f8 or l3f8:
                    wtf = w23fp.tile([128, 8, 128], F8, tag="wf")
                if not (l2f8 and l3f8):
                    wtb = w23bp.tile([128, 1024], BF16, tag="wb")
                w0 = st * 1024
                if l2f8:
                    nc.sync.dma_start(wtf[:, 0:4, :], w23f_d[:, w0 : w0 + 512])
                else:
                    nc.sync.dma_start(wtb[:, 0:512], w23b_d[:, w0 : w0 + 512])
                if l3f8:
                    nc.sync.dma_start(wtf[:, 4:8, :],
                                      w23f_d[:, w0 + 512 : w0 + 1024])
                else:
                    nc.sync.dma_start(wtb[:, 512:1024],
                                      w23b_d[:, w0 + 512 : w0 + 1024])
                return (wt1, wtb, wtf)

            zt = zp.tile([128, half], BF16, tag="z")
            for m in range(nmacro):
                nc.gpsimd.dma_start(zt[:, bass.ts(m, FD)], xt[:, bass.ts(m, FD)])

            wts = {0: fetch_weights(0)}
            if nsteps > 1:
                wts[1] = fetch_weights(1)

            def stageA_mm(st, m):
                """L1 (bf16): 4 block-diag MMs into two 2-bank psum tiles
                (h1a = half A [k0|k1], h1b = half B) so the next macro's L1
                only waits on the matching half's evacuation."""
                wt1 = wts[st][0]
                zsl = zt[:, bass.ts(m, FD)]
                h1a = pH1.tile([128, 2, FD], F32, tag="h1a")
                h1b = pH1.tile([128, 2, FD], F32, tag="h1b")
                nc.tensor.matmul(h1a[:, 0, :], wt1[:, 0:128], zsl)
                nc.tensor.matmul(h1b[:, 0, :], wt1[:, 128:256], zsl)
                nc.tensor.matmul(h1a[:, 1, :], wt1[:, 256:384], zsl)
                nc.tensor.matmul(h1b[:, 1, :], wt1[:, 384:512], zsl)
                return h1a, h1b

            def stageA_relu(st, m, h1ps):
                """h1 evacuation: half A on ACT, half B on DVE (emitted
                after stage B so engine queue order matches dependency
                order)."""
                l2f8 = cfg[st][0]
                h1a, h1b = h1ps
                if l2f8:
                    h1wA = h1p.tile([128, 2, FD], F8, tag="h1w8A")
                    h1wB = h1p.tile([128, 2, FD], F8, tag="h1w8B")
                else:
                    h1wA = h1p.tile([128, 2 * FD], BF16, tag="h1wA")
                    h1wB = h1p.tile([128, 2 * FD], BF16, tag="h1wB")
                nc.scalar.activation(h1wA[:], h1a[:], AF.Relu)
                nc.vector.tensor_scalar(h1wB[:], h1b[:], 0.0, None, op0=ALU.max)
                return h1wA, h1wB

            def stageB(st, m, h1w):
                """L2 + h2 relus + L3 + z add, per-step precision.
                h1w layout: subtiles [A-k0, A-k1, B-k0, B-k1]."""
                l2f8, l3f8 = cfg[st]
                _, wtb, wtf = wts[st]
                zsl = zt[:, bass.ts(m, FD)]
                h1wA, h1wB = h1w
                if l2f8:
                    h1A, h1B = h1wA[:], h1wB[:]
                else:
                    h1A = (h1wA[:, 0:FD], h1wA[:, FD : 2 * FD])
                    h1B = (h1wB[:, 0:FD], h1wB[:, FD : 2 * FD])

                h2A = pH2.tile([128, 2 * FD], F32, tag="h2A")
                h2B = pH2.tile([128, FD], F32, tag="h2B")
                dps = pD.tile([128, FD], F32, tag="dps")

                if l3f8:
                    h2wA = h2p.tile([128, 2, FD], F8, tag="h2w8A")
                    h2wB = h2p.tile([128, 2, FD], F8, tag="h2w8B")
                    h2wA_f, h2wB_f = h2wA[:], h2wB[:]
                    h2wA_m = (h2wA[:, 0, :], h2wA[:, 1, :])
                    h2wB_m = (h2wB[:, 0, :], h2wB[:, 1, :])
                else:
                    h2wA = h2p.tile([128, 2 * FD], BF16, tag="h2wA")
                    h2wB = h2p.tile([128, 2 * FD], BF16, tag="h2wB")
                    h2wA_f, h2wB_f = h2wA[:], h2wB[:]
                    h2wA_m = (h2wA[:, 0:FD], h2wA[:, FD : 2 * FD])
                    h2wB_m = (h2wB[:, 0:FD], h2wB[:, FD : 2 * FD])

                # --- L2 matmuls ---
                if l2f8:
                    nc.tensor.matmul(h2A[:, 0:FD], wtf[:, 0:2, :], h1A,
                                     perf_mode=DRMODE)
                    nc.tensor.matmul(h2A[:, FD : 2 * FD], wtf[:, 2:4, :],
                                     h1A, perf_mode=DRMODE)
                    nc.tensor.matmul(h2B[:], wtf[:, 0:2, :], h1B,
                                     perf_mode=DRMODE)
                    # half-B mc1 goes to the pD bank (L3 reuses it after)
                    nc.tensor.matmul(dps[:], wtf[:, 2:4, :], h1B,
                                     perf_mode=DRMODE)
                else:
                    w2 = [wtb[:, 128 * i : 128 * (i + 1)] for i in range(4)]
                    nc.tensor.matmul(h2A[:, 0:FD], w2[0], h1A[0],
                                     start=True, stop=False)
                    nc.tensor.matmul(h2A[:, 0:FD], w2[1], h1A[1],
                                     start=False, stop=True)
                    nc.tensor.matmul(h2A[:, FD : 2 * FD], w2[2], h1A[0],
                                     start=True, stop=False)
                    nc.tensor.matmul(h2A[:, FD : 2 * FD], w2[3], h1A[1],
                                     start=False, stop=True)
                    nc.tensor.matmul(h2B[:], w2[0], h1B[0],
                                     start=True, stop=False)
                    nc.tensor.matmul(h2B[:], w2[1], h1B[1],
                                     start=False, stop=True)
                    nc.tensor.matmul(dps[:], w2[2], h1B[0],
                                     start=True, stop=False)
                    nc.tensor.matmul(dps[:], w2[3], h1B[1],
                                     start=False, stop=True)

                # --- h2 relus: ACT gets A (merged) + B-mc1, DVE gets B-mc0.
                # (Splitting the B-mc1 evac across ACT+DVE to rebalance was
                # tried and regressed 35% — two engines reading the same
                # PSUM bank serializes the pipeline.) ---
                nc.scalar.activation(h2wA_f, h2A[:], AF.Relu)
                nc.vector.tensor_scalar(h2wB_m[0], h2B[:], 0.0, None,
                                        op0=ALU.max)
                nc.scalar.activation(h2wB_m[1], dps[:], AF.Relu)

                # --- L3 matmuls (dps bank reused after B-mc1 evacuated) ---
                if l3f8:
                    nc.tensor.matmul(dps[:], wtf[:, 4:6, :], h2wA_f,
                                     start=True, stop=False, perf_mode=DRMODE)
                    nc.tensor.matmul(dps[:], wtf[:, 6:8, :], h2wB_f,
                                     start=False, stop=True, perf_mode=DRMODE)
                else:
                    w3 = [wtb[:, 512 + 128 * i : 640 + 128 * i] for i in range(4)]
                    nc.tensor.matmul(dps[:], w3[0], h2wA_m[0],
                                     start=True, stop=False)
                    nc.tensor.matmul(dps[:], w3[1], h2wA_m[1],
                                     start=False, stop=False)
                    nc.tensor.matmul(dps[:], w3[2], h2wB_m[0],
                                     start=False, stop=False)
                    nc.tensor.matmul(dps[:], w3[3], h2wB_m[1],
                                     start=False, stop=True)

                def tail(dps=dps, zsl=zsl, st=st, m=m):
                    # zadd emitted after the next macro's h1 relus so the
                    # DVE queue order (h1wB first) matches dependency order
                    nc.vector.tensor_add(zsl, dps[:], zsl)
                    if st == nsteps - 1:
                        ostage = op.tile([128, FD], BF16, tag="ostage")
                        nc.vector.tensor_scalar_mul(ostage[:], zsl, exps[:])
                        nc.sync.dma_start(out_d[:, bass.ts(m, FD)], ostage[:])
                return tail

            # Pipeline emission per macro: [L1-mms(m)] [stageB-front(m-1)]
            # [h1-relus(m)] [zadd(m-1)] so each engine's queue order matches
            # dependency readiness order.
            # Depth-8 step-wavefront order (slots sorted by m + 2*st): ~8
            # consecutive steps' macros interleave so PE-bound (L3-bf16)
            # and ACT-bound (L3-fp8) step types average across the engine
            # queues instead of each stranding the other engine. (st+1, m)
            # trails (st, m) by ~16 slots, far above the 2-slot pipeline
            # minimum required for the z-slice dependency.
            flat = [(st, m) for st in range(nsteps) for m in range(nmacro)]
            flat.sort(key=lambda sm: (sm[1] + 2 * sm[0], sm[0]))
            pending = None  # (st, m, h1w) awaiting stageB
            next_w = 2
            for k, (st, m) in enumerate(flat):
                wave = m + 2 * st
                while next_w < nsteps and 2 * next_w <= wave + 6:
                    wts[next_w] = fetch_weights(next_w)
                    next_w += 1
                h1ps = stageA_mm(st, m)
                tail = None
                if pending is not None:
                    pst, pm, ph1w = pending
                    tail = stageB(pst, pm, ph1w)
                h1w = stageA_relu(st, m, h1ps)
                if tail is not None:
                    tail()
                pending = (st, m, h1w)
            pst, pm, ph1w = pending
            stageB(pst, pm, ph1w)()

    nc.finalize()
    return nc


def build_program_v2(nsteps=S, nmacro=NMACRO):
    """v3: all matmuls in uniform 128x128 array mode (no tiling-mode drains).

    L1 uses block-diagonal weight tiles ([W1e;0] / [0;W1e]) so each half's
    h1 comes from a full-K matmul over the shared z tile. L3 uses M=128
    zero-padded W3e tiles so both halves' updates accumulate into one PSUM
    bank. Emission is software-pipelined: stage A (L1+relu) runs one macro
    ahead of stage B (L2+relu+L3+zadd). Weights stream per step through a
    rotating pool with 2-step prefetch.
    """
    half = nmacro * FD
    nc = bacc.Bacc("TRN2", target_bir_lowering=False, debug=False)

    xt = nc.dram_tensor("xt", [128, half], BF16, kind="ExternalInput")
    wq = nc.dram_tensor("wq", [128, nsteps * WCOLS], BF16, kind="ExternalInput")
    s_d = nc.dram_tensor("sv", [128, 1], F32, kind="ExternalInput")
    out_d = nc.dram_tensor("out", [128, half], F32, kind="ExternalOutput")

    with TileContext(nc) as tc:
        with (
            tc.tile_pool(name="zpool", bufs=1) as zp,
            tc.tile_pool(name="consts", bufs=1) as cp,
            tc.tile_pool(name="wpool", bufs=4) as wp,
            tc.tile_pool(name="h1pool", bufs=4) as h1p,
            tc.tile_pool(name="h2pool", bufs=4) as h2p,
            tc.tile_pool(name="opool", bufs=3) as op,
            tc.tile_pool(name="psH1", bufs=1, space="PSUM") as pH1,
            tc.tile_pool(name="psH2", bufs=1, space="PSUM") as pH2,
            tc.tile_pool(name="psD", bufs=1, space="PSUM") as pD,
        ):
            # --- constants ---
            ss = cp.tile([128, 1], F32, tag="ss")
            nc.sync.dma_start(ss[:], s_d[:])
            exps = cp.tile([128, 1], F32, tag="exps")
            nc.scalar.activation(exps[:], ss[:], AF.Exp)

            def fetch_weights(st):
                wt = wp.tile([128, WCOLS], BF16, tag="w")
                nc.sync.dma_start(wt[:], wq[:, bass.ts(st, WCOLS)])
                return wt

            # --- z state, resident in SBUF ---
            zt = zp.tile([128, half], BF16, tag="z")
            for m in range(nmacro):
                nc.gpsimd.dma_start(zt[:, bass.ts(m, FD)], xt[:, bass.ts(m, FD)])

            wts = {0: fetch_weights(0)}
            if nsteps > 1:
                wts[1] = fetch_weights(1)

            def stageA(st, m):
                """L1 for (st, m): 4 block-diag MMs + h1 relus."""
                wt = wts[st]
                zsl = zt[:, bass.ts(m, FD)]
                h1a = pH1.tile([128, 2 * FD], F32, tag="h1a")
                h1b = pH1.tile([128, 2 * FD], F32, tag="h1b")
                # tiles: [W1e_c0;0] [0;W1e_c0] [W1e_c1;0] [0;W1e_c1]
                nc.tensor.matmul(h1a[:, 0:FD], wt[:, 0:128], zsl)
                nc.tensor.matmul(h1b[:, 0:FD], wt[:, 128:256], zsl)
                nc.tensor.matmul(h1a[:, FD : 2 * FD], wt[:, 256:384], zsl)
                nc.tensor.matmul(h1b[:, FD : 2 * FD], wt[:, 384:512], zsl)
                h1wA = h1p.tile([128, 2 * FD], BF16, tag="h1wA")
                h1wB = h1p.tile([128, 2 * FD], BF16, tag="h1wB")
                nc.scalar.activation(h1wA[:], h1a[:], AF.Relu)
                nc.vector.tensor_scalar(h1wB[:], h1b[:], 0.0, None, op0=ALU.max)
                return h1wA, h1wB

            def stageB(st, m, h1w):
                """L2 + h2 relus + L3 (M=128-padded, one bank) + z add."""
                wt = wts[st]
                h1wA, h1wB = h1w
                zsl = zt[:, bass.ts(m, FD)]
                w2 = [wt[:, 512 + 128 * i : 640 + 128 * i] for i in range(4)]
                w3 = [wt[:, 1024 + 128 * i : 1152 + 128 * i] for i in range(4)]

                h2ts = []
                h2w = {}
                for hi, (hw, hx) in enumerate(((h1wA, "A"), (h1wB, "B"))):
                    for mc in range(2):
                        if hi == 1 and mc == 1:
                            ps = h2ts[0]  # reuse A-m0 bank (PSUM budget)
                        else:
                            ps = pH2.tile([128, FD], F32, tag=f"h2_{len(h2ts)}")
                            h2ts.append(ps)
                        nc.tensor.matmul(
                            ps[:], w2[2 * mc], hw[:, 0:FD], start=True, stop=False
                        )
                        nc.tensor.matmul(
                            ps[:], w2[2 * mc + 1], hw[:, FD : 2 * FD],
                            start=False, stop=True,
                        )
                        if mc == 0:
                            h2wt = h2p.tile([128, 2 * FD], BF16, tag=f"h2w{hx}")
                            h2w[hx] = h2wt
                        osl = h2w[hx][:, mc * FD : (mc + 1) * FD]
                        if hi == 1 and mc == 1:
                            nc.vector.tensor_scalar(
                                osl, ps[:], 0.0, None, op0=ALU.max
                            )
                        else:
                            nc.scalar.activation(osl, ps[:], AF.Relu)

                dps = pD.tile([128, FD], F32, tag="dps")
                nc.tensor.matmul(
                    dps[:], w3[0], h2w["A"][:, 0:FD], start=True, stop=False
                )
                nc.tensor.matmul(
                    dps[:], w3[1], h2w["A"][:, FD : 2 * FD], start=False, stop=False
                )
                nc.tensor.matmul(
                    dps[:], w3[2], h2w["B"][:, 0:FD], start=False, stop=False
                )
                nc.tensor.matmul(
                    dps[:], w3[3], h2w["B"][:, FD : 2 * FD], start=False, stop=True
                )
                nc.vector.tensor_add(zsl, dps[:], zsl)
                if st == nsteps - 1:
                    # last step: scale + store this macro right away so the
                    # output phase overlaps the remaining macros' compute
                    ostage = op.tile([128, FD], F32, tag="ostage")
                    nc.vector.tensor_scalar_mul(ostage[:], zsl, exps[:])
                    nc.sync.dma_start(out_d[:, bass.ts(m, FD)], ostage[:])

            # --- software-pipelined main loop: A one macro ahead of B ---
            flat = [(st, m) for st in range(nsteps) for m in range(nmacro)]
            pending = None  # (st, m, h1w)
            for k, (st, m) in enumerate(flat):
                # prefetch next step's weights mid-step
                if m == nmacro // 2 and st + 2 < nsteps:
                    wts[st + 2] = fetch_weights(st + 2)
                h1w = stageA(st, m)
                if pending is not None:
                    pst, pm, ph1w = pending
                    stageB(pst, pm, ph1w)
                pending = (st, m, h1w)
            pst, pm, ph1w = pending
            stageB(pst, pm, ph1w)

    nc.finalize()
    return nc


def host_prep_v4(x, s, W1, W2, W3, idx):
    """v4 weight packing: L1 bf16 tensor + L2/L3 in both bf16 and fp8."""
    import ml_dtypes

    wq, _ = _host_pack_f32(W1, W2, W3, idx)
    sv = np.exp(np.asarray(s, np.float64)).astype(np.float32).reshape(N, 1)
    sv = np.ascontiguousarray(np.concatenate([sv, sv], axis=0))  # [128, 1]
    wb = np.empty((128, S * 512), np.float32)
    w23 = np.empty((128, S * 1024), np.float32)
    for st in range(S):
        wb[:, st * 512 : (st + 1) * 512] = wq[:, st * WCOLS : st * WCOLS + 512]
        w23[:, st * 1024 : (st + 1) * 1024] = wq[
            :, st * WCOLS + 512 : (st + 1) * WCOLS
        ]
    return (
        wb.astype(ml_dtypes.bfloat16),
        w23.astype(ml_dtypes.bfloat16),
        w23.astype(ml_dtypes.float8_e4m3),
        sv,
    )


def _host_pack_f32(W1, W2, W3, idx):
    """Shared f32 packing (v3 wq layout)."""
    W1 = np.asarray(W1, np.float32)
    W2 = np.asarray(W2, np.float32)
    W3 = np.asarray(W3, np.float32)
    idx = np.asarray(idx)

    wq = np.zeros((128, S * WCOLS), np.float32)
    for st in range(S):
        i = int(idx[st])
        W1e = np.zeros((N, H), np.float32)
        W1e[: N - 1] = W1[st]
        W1e[[i, N - 1]] = W1e[[N - 1, i]]
        W3e = np.zeros((H, N), np.float32)
        W3e[:, i] = W3[st, :, 0]
        w0 = st * WCOLS
        wq[0:64, w0 + 0 : w0 + 128] = W1e[:, 0:128]
        wq[64:128, w0 + 128 : w0 + 256] = W1e[:, 0:128]
        wq[0:64, w0 + 256 : w0 + 384] = W1e[:, 128:256]
        wq[64:128, w0 + 384 : w0 + 512] = W1e[:, 128:256]
        wq[:, w0 + 512 : w0 + 640] = W2[st, 0:128, 0:128]
        wq[:, w0 + 640 : w0 + 768] = W2[st, 128:256, 0:128]
        wq[:, w0 + 768 : w0 + 896] = W2[st, 0:128, 128:256]
        wq[:, w0 + 896 : w0 + 1024] = W2[st, 128:256, 128:256]
        wq[:, w0 + 1024 : w0 + 1088] = W3e[0:128, :]
        wq[:, w0 + 1152 : w0 + 1216] = W3e[128:256, :]
        wq[:, w0 + 1344 : w0 + 1408] = W3e[0:128, :]
        wq[:, w0 + 1472 : w0 + 1536] = W3e[128:256, :]
    sv = None
    return wq, sv


def host_prep_v2(x, s, W1, W2, W3, idx):
    """Build per-step packed weights and the split-sample transposed x."""
    import ml_dtypes

    W1 = np.asarray(W1, np.float32)
    W2 = np.asarray(W2, np.float32)
    W3 = np.asarray(W3, np.float32)
    idx = np.asarray(idx)

    wq = np.zeros((128, S * WCOLS), np.float32)
    for st in range(S):
        i = int(idx[st])
        W1e = np.zeros((N, H), np.float32)
        W1e[: N - 1] = W1[st]
        W1e[[i, N - 1]] = W1e[[N - 1, i]]
        W3e = np.zeros((H, N), np.float32)
        W3e[:, i] = W3[st, :, 0]
        w0 = st * WCOLS
        # L1 block-diagonal tiles: [W1e_c;0] for half A, [0;W1e_c] for B
        wq[0:64, w0 + 0 : w0 + 128] = W1e[:, 0:128]
        wq[64:128, w0 + 128 : w0 + 256] = W1e[:, 0:128]
        wq[0:64, w0 + 256 : w0 + 384] = W1e[:, 128:256]
        wq[64:128, w0 + 384 : w0 + 512] = W1e[:, 128:256]
        # W2 tiles (k-chunk-major within each m-chunk)
        wq[:, w0 + 512 : w0 + 640] = W2[st, 0:128, 0:128]
        wq[:, w0 + 640 : w0 + 768] = W2[st, 128:256, 0:128]
        wq[:, w0 + 768 : w0 + 896] = W2[st, 0:128, 128:256]
        wq[:, w0 + 896 : w0 + 1024] = W2[st, 128:256, 128:256]
        # W3e M=128-padded tiles: cols 0:64 update half A, 64:128 half B
        wq[:, w0 + 1024 : w0 + 1088] = W3e[0:128, :]
        wq[:, w0 + 1152 : w0 + 1216] = W3e[128:256, :]
        wq[:, w0 + 1344 : w0 + 1408] = W3e[0:128, :]
        wq[:, w0 + 1472 : w0 + 1536] = W3e[128:256, :]
    wq = wq.astype(ml_dtypes.bfloat16)

    sv = np.asarray(s, np.float32).reshape(N, 1)
    sv = np.concatenate([sv, sv], axis=0)  # [128, 1]
    return wq, sv


_PROGRAM_V2 = None
_RUN_IDX = 0


def kernel(x, s, W1, b1, W2, b2, W3, b3, idx):
    global LAST_RESULT, _PROGRAM_V2
    use_bias = bool(
        np.abs(b1).max() > 0 or np.abs(b2).max() > 0 or np.abs(b3).max() > 0
    )
    if use_bias:
        return _kernel_v1(x, s, W1, b1, W2, b2, W3, b3, idx)

    x = np.asarray(x, np.float32)
    wb, w23b, w23f, sv = host_prep_v4(x, s, W1, W2, W3, idx)
    in_maps = []
    for c in range(NCORES):
        xc = x[c * BSH : (c + 1) * BSH]
        xts = np.empty((128, HALF), np.float32)
        xts[0:64] = xc[0:HALF].T
        xts[64:128] = xc[HALF:BSH].T
        import ml_dtypes
        in_maps.append(
            dict(xt=np.ascontiguousarray(xts).astype(ml_dtypes.bfloat16),
                 wb=wb, w23b=w23b, w23f=w23f, sv=sv)
        )

    if _PROGRAM_V2 is None:
        _PROGRAM_V2 = build_program_v4()
    _ensure_ntff_hook()
    global _RUN_IDX
    tmpdir = os.environ.get("KERNEL_TMPDIR")
    if tmpdir:
        tmpdir = os.path.join(tmpdir, f"run{_RUN_IDX}")
        _RUN_IDX += 1
        os.makedirs(tmpdir, exist_ok=True)
    res = run_bass_kernel_spmd(
        _PROGRAM_V2, in_maps, core_ids=list(range(NCORES)), tmpdir=tmpdir
    )
    LAST_RESULT = res
    out = np.empty((B, N), np.float32)
    for c in range(NCORES):
        o = np.asarray(res.results[c]["out"]).astype(np.float32)  # [128, HALF]
        out[c * BSH : c * BSH + HALF] = o[0:64].T
        out[c * BSH + HALF : (c + 1) * BSH] = o[64:128].T
    return out


# ---------------------------------------------------------------------------
# v1 fallback (baseline) — used only when biases are nonzero.
# ---------------------------------------------------------------------------
TILE = 512
MACRO = 1024
_PROGRAM_V1 = {}


def build_program_v1(nsteps=S, nmacro=BSH // MACRO, use_bias=True, hbufs=3):
    bsh = nmacro * MACRO
    nc = bacc.Bacc("TRN2", target_bir_lowering=False, debug=False)

    xt = nc.dram_tensor("xt", [N, bsh], BF16, kind="ExternalInput")
    wp_d = nc.dram_tensor("wpack", [nsteps, 128, 896], BF16, kind="ExternalInput")
    b1_d = nc.dram_tensor("b1r", [128, 2 * nsteps], F32, kind="ExternalInput")
    b2_d = nc.dram_tensor("b2r", [128, 2 * nsteps], F32, kind="ExternalInput")
    b3_d = nc.dram_tensor("b3c", [N, nsteps], F32, kind="ExternalInput")
    s_d = nc.dram_tensor("sv", [N, 1], F32, kind="ExternalInput")
    out_d = nc.dram_tensor("out", [N, bsh], F32, kind="ExternalOutput")

    with TileContext(nc) as tc:
        with (
            tc.tile_pool(name="zpool", bufs=1) as zp,
            tc.tile_pool(name="consts", bufs=1) as cp,
            tc.tile_pool(name="wpool", bufs=4) as wp,
            tc.tile_pool(name="hpool", bufs=hbufs) as hp,
            tc.tile_pool(name="psA", bufs=3, space="PSUM") as pA,
            tc.tile_pool(name="psB", bufs=3, space="PSUM") as pB,
            tc.tile_pool(name="psZ", bufs=2, space="PSUM") as pZ,
        ):
            if use_bias:
                b1s = cp.tile([128, 2 * nsteps], F32, tag="b1s")
                nc.sync.dma_start(b1s[:], b1_d[:])
                b2s = cp.tile([128, 2 * nsteps], F32, tag="b2s")
                nc.sync.dma_start(b2s[:], b2_d[:])
                b3s = cp.tile([N, nsteps], F32, tag="b3s")
                nc.sync.dma_start(b3s[:], b3_d[:])
            ss = cp.tile([N, 1], F32, tag="ss")
            nc.sync.dma_start(ss[:], s_d[:])
            exps = cp.tile([N, 1], F32, tag="exps")
            nc.scalar.activation(exps[:], ss[:], AF.Exp)

            def fetch_weights(st):
                wt = wp.tile([128, 896], BF16, tag="w")
                nc.sync.dma_start(wt[:], wp_d[st])
                return (
                    wt[0:N, 0:H], wt[:, 256:512], wt[:, 512:768],
                    wt[:, 768:832], wt[:, 832:896],
                )

            wtiles = fetch_weights(0)
            zt = zp.tile([N, bsh], BF16, tag="z")
            for m in range(nmacro):
                msl = bass.ts(m, MACRO)
                nc.gpsimd.dma_start(zt[:, msl], xt[:, msl])

            pending_l3 = None
            for st in range(nsteps):
                if st > 0:
                    wtiles = fetch_weights(st)
                w1t, w2ta, w2tb, w3ta, w3tb = wtiles

                for m in range(nmacro):
                    zsl = zt[:, bass.ts(m, MACRO)]

                    def act_relu(out, in_, bcol):
                        if use_bias:
                            nc.scalar.activation(out, in_, AF.Relu, bias=bcol)
                        else:
                            nc.scalar.activation(out, in_, AF.Relu)

                    def dve_relu(out, in_, bcol):
                        if use_bias:
                            nc.vector.tensor_scalar(
                                out, in_, bcol, 0.0, op0=ALU.add, op1=ALU.max
                            )
                        else:
                            nc.vector.tensor_scalar(out, in_, 0.0, None, op0=ALU.max)

                    b1a = b1s[:, 2 * st : 2 * st + 1] if use_bias else None
                    b1b = b1s[:, 2 * st + 1 : 2 * st + 2] if use_bias else None
                    b2a = b2s[:, 2 * st : 2 * st + 1] if use_bias else None
                    b2b = b2s[:, 2 * st + 1 : 2 * st + 2] if use_bias else None

                    h1ps = []
                    for t in range(MACRO // TILE):
                        tsl = bass.ts(t, TILE)
                        pa = pA.tile([128, TILE], F32, tag="h1p")
                        pb = pA.tile([128, TILE], F32, tag="h1p")
                        nc.tensor.matmul(pa[:], w1t[:, 0:128], zsl[:, tsl])
                        nc.tensor.matmul(pb[:], w1t[:, 128:256], zsl[:, tsl])
                        h1ps.append((pa, pb))
                    if pending_l3 is not None:
                        pending_l3()
                        pending_l3 = None
                    h1a = hp.tile([128, MACRO], BF16, tag="h1a")
                    h1b = hp.tile([128, MACRO], BF16, tag="h1b")
                    act_relu(h1a[:, 0:TILE], h1ps[0][0][:], b1a)
                    dve_relu(h1b[:, 0:TILE], h1ps[0][1][:], b1b)
                    act_relu(h1a[:, TILE:MACRO], h1ps[1][0][:], b1a)
                    act_relu(h1b[:, TILE:MACRO], h1ps[1][1][:], b1b)

                    h2a = hp.tile([128, MACRO], BF16, tag="h2a")
                    h2b = hp.tile([128, MACRO], BF16, tag="h2b")
                    for t in range(MACRO // TILE):
                        tsl = bass.ts(t, TILE)
                        pa = pB.tile([128, TILE], F32, tag="h2p")
                        pb = pB.tile([128, TILE], F32, tag="h2p")
                        nc.tensor.matmul(
                            pa[:], w2ta[:, 0:128], h1a[:, tsl], start=True, stop=False
                        )
                        nc.tensor.matmul(
                            pa[:], w2tb[:, 0:128], h1b[:, tsl], start=False, stop=True
                        )
                        nc.tensor.matmul(
                            pb[:], w2ta[:, 128:256], h1a[:, tsl], start=True, stop=False
                        )
                        nc.tensor.matmul(
                            pb[:], w2tb[:, 128:256], h1b[:, tsl], start=False, stop=True
                        )
                        act_relu(h2a[:, tsl], pa[:], b2a)
                        dve_relu(h2b[:, tsl], pb[:], b2b)

                    def emit_l3(h2a=h2a, h2b=h2b, zsl=zsl, w3ta=w3ta, w3tb=w3tb, st=st):
                        for t in range(MACRO // TILE):
                            tsl = bass.ts(t, TILE)
                            zps = pZ.tile([N, TILE], F32, tag="zp")
                            nc.tensor.matmul(
                                zps[:], w3ta[:], h2a[:, tsl], start=True, stop=False
                            )
                            nc.tensor.matmul(
                                zps[:], w3tb[:], h2b[:, tsl], start=False, stop=True
                            )
                            ztile = zsl[:, tsl]
                            if use_bias:
                                nc.vector.scalar_tensor_tensor(
                                    ztile, zps[:], b3s[:, st : st + 1], ztile,
                                    op0=ALU.add, op1=ALU.add,
                                )
                            else:
                                nc.vector.tensor_add(ztile, zps[:], ztile)

                    pending_l3 = emit_l3

            if pending_l3 is not None:
                pending_l3()

            for m in range(nmacro):
                msl = bass.ts(m, MACRO)
                ostage = hp.tile([N, MACRO], F32, tag="ostage")
                nc.vector.tensor_scalar_mul(ostage[:], zt[:, msl], exps[:])
                nc.sync.dma_start(out_d[:, msl], ostage[:])

    nc.finalize()
    return nc


def _host_prep_v1(x, s, W1, b1, W2, b2, W3, b3, idx, nsteps=S):
    x = np.asarray(x, np.float32)
    idx = np.asarray(idx)
    W1 = np.asarray(W1, np.float32)
    W2 = np.ascontiguousarray(np.asarray(W2, np.float32)[:nsteps])
    W3 = np.asarray(W3, np.float32)
    b1 = np.asarray(b1, np.float32)
    b2 = np.asarray(b2, np.float32)
    b3 = np.asarray(b3, np.float32)

    W1e = np.zeros((nsteps, N, H), np.float32)
    W1e[:, : N - 1, :] = W1[:nsteps]
    for st in range(nsteps):
        i = int(idx[st])
        r = W1e[st].copy()
        r[[i, N - 1]] = r[[N - 1, i]]
        W1e[st] = r
    W3e = np.zeros((nsteps, H, N), np.float32)
    for st in range(nsteps):
        W3e[st, :, int(idx[st])] = W3[st, :, 0]
    b3c = np.zeros((N, nsteps), np.float32)
    for st in range(nsteps):
        b3c[int(idx[st]), st] = b3[st, 0]
    import ml_dtypes
    wpack = np.zeros((nsteps, 128, 896), np.float32)
    wpack[:, 0:N, 0:H] = W1e
    wpack[:, :, 256:512] = W2[:, 0:128, :]
    wpack[:, :, 512:768] = W2[:, 128:256, :]
    wpack[:, :, 768:832] = W3e[:, 0:128, :]
    wpack[:, :, 832:896] = W3e[:, 128:256, :]
    b1r = np.ascontiguousarray(
        b1[:nsteps].reshape(nsteps, 2, 128).transpose(2, 0, 1).reshape(128, 2 * nsteps)
    )
    b2r = np.ascontiguousarray(
        b2[:nsteps].reshape(nsteps, 2, 128).transpose(2, 0, 1).reshape(128, 2 * nsteps)
    )
    wpack = wpack.astype(ml_dtypes.bfloat16)
    xt = np.ascontiguousarray(x.T).astype(ml_dtypes.bfloat16)
    sv = np.ascontiguousarray(np.asarray(s, np.float32).reshape(N, 1))
    return dict(wpack=wpack, b1r=b1r, b2r=b2r, b3c=b3c, sv=sv), xt


def _kernel_v1(x, s, W1, b1, W2, b2, W3, b3, idx):
    global LAST_RESULT
    shared, xt = _host_prep_v1(x, s, W1, b1, W2, b2, W3, b3, idx)
    in_maps = []
    for c in range(NCORES):
        m = dict(shared)
        m["xt"] = np.ascontiguousarray(xt[:, c * BSH : (c + 1) * BSH])
        in_maps.append(m)
    if True not in _PROGRAM_V1:
        _PROGRAM_V1[True] = build_program_v1(use_bias=True)
    _ensure_ntff_hook()
    res = run_bass_kernel_spmd(
        _PROGRAM_V1[True], in_maps, core_ids=list(range(NCORES))
    )
    LAST_RESULT = res
    outs = [res.results[c]["out"] for c in range(NCORES)]
    return np.ascontiguousarray(
        np.concatenate([o.T for o in outs], axis=0), dtype=np.float32
    )



# revision 32
# speedup vs baseline: 1.0125x; 1.0125x over previous
"""Trainium2 Bass kernel for the additive coupling flow (nn_Additive_flow).

Math: 65 sequential steps. Step s (i = idx[s]) updates column i of z:
    z[:, i] += MLP_s(z with cols i<->63 swapped, first 63 cols) + b3[s]
Reformulated with no data permutation:
    h1 = relu(z @ W1e[s])      W1e[s] = [W1[s]; 0] with rows i,63 swapped
    h2 = relu(h1 @ W2[s])
    z[:, i] += h2 @ w3[s]      (plus biases; zero in practice)
Finally out = exp(s_vec) * z.

Device layout ("split-sample"): each core's 16384 samples are split in two
halves of 8192; SBUF state z is [128, 8192] bf16 where partitions 0:63
hold features of half A and 64:127 of half B.

v4+ mixed precision (fp8 DoubleRow): a DR matmul contracts K=256 (two
128-row subtiles) in the same wall-time as one K=128 bf16 matmul
(measured 221.6 ns per 512-col mm either way) = 2x MAC rate. Per-step
schedule from a numpy error study (sim transfers to HW within ~2%):
L2 runs fp8-DR except the 4 most error-sensitive steps {26,27,50,51}
(which alone carry ~50% of the full-fp8 variance); L3 runs fp8-DR on 41
cheap steps; L1 stays bf16 (z would need an extra fp8 cast + error).
Weight-side hi/lo fp8 tricks are pointless on real HW (2-term = bf16
cost), activation-side hi/lo loses to the elementwise wall.
  - L1 (bf16): block-diagonal weight tiles [W1e_c;0]/[0;W1e_c], 4
    MMs/macro into two 2-bank PSUM tiles (h1a=half A [k0|k1], h1b=B).
  - L2 fp8: 4 DR MMs/macro (A-mc0, A-mc1, B-mc0, B-mc1->pD bank);
    bf16 fallback: 8 MMs.
  - L3 fp8: 2 DR MMs accumulating into the pD bank (reused after the
    B-mc1 evacuation); bf16: 4 MMs with M=128 zero-padded scatter tiles.
  - relu+dtype-convert is free in the evacuation op: ACT does h1a
    [128,1024], h2A [128,1024], h2B-mc1 [128,512] (2.69 us/macro); DVE
    does h1b, h2B-mc0, zadd (2.51 us/macro). The PSUM->SBUF elementwise
    wall (4608 el/partition/macro; Pool engine cannot access PSUM) is the
    binding constraint together with ACT-PE dependency alignment.
Emission per macro: [L1-mms(m)] [stageB-front(m-1)] [h1-relus(m)]
[zadd(m-1)]. PSUM: h1 4 banks + h2A 2 + h2B 1 + pD 1 = 8 exactly.
Weights stream per step (bf16 L1 tensor + bf16/fp8 L2L3 tensors, only
the needed slices DMA'd) with 2-step prefetch. exp(s) is precomputed on
host (device Exp table costs ~1e-3 noise); output is stored/DMA'd in
bf16 (halves output traffic, +0.01e-2 err). Measured: 3.134 ms at rel
err 1.61e-2 (budget 2e-2) vs 3.707 ms bf16 baseline. Per-macro steady
state: ACT busy 2697 ns (binding on L3-fp8 steps, +~140 ns semaphore
tail), PE 2678 avg (binding on L3-bf16 steps), DVE 2454. Measured dead
ends: walrus --enable-ldw-opt=true crashes codegen (would reclaim
~250us of LDWEIGHTS); splitting one PSUM region's evacuation across
ACT+DVE serializes (-35%); monolithic 4-bank h1 psum tile serializes
on the whole-tile WAR (-28%). Note: the device occasionally lands in a
transient ~+0.6ms slow mode on a fresh process; rerunning recovers.
"""

import os
import sys

for _p in ("/opt/trn_rl_repo", "/root/.axon_site/_ro/trn_rl_repo"):
    if os.path.isdir(_p) and _p not in sys.path:
        sys.path.append(_p)

import numpy as np
import concourse.bass as bass
import concourse.bacc as bacc
import concourse.mybir as mybir
from concourse.tile import TileContext
from concourse.bass_utils import run_bass_kernel_spmd

NCORES = 8
B = 131072
N = 64          # latent dim
S = 65          # coupling steps
H = 256         # MLP width
BSH = B // NCORES      # 16384 samples per core
HALF = BSH // 2        # 8192 samples per partition-half
FD = 512               # matmul moving free-dim; one macro = 2*FD samples
NMACRO = HALF // FD    # 16
WCOLS = 1536           # per-step packed weight columns

F32 = mybir.dt.float32
F32R = mybir.dt.float32r
BF16 = mybir.dt.bfloat16
AF = mybir.ActivationFunctionType
ALU = mybir.AluOpType

LAST_RESULT = None  # test.py reads exec_time_ns from here

_HOOK_SRC = """\
_hook = None


def set_axon_ntff_profile_hook(h):
    global _hook
    _hook = h


def get_axon_ntff_profile_hook():
    return _hook
"""


def _ensure_ntff_hook():
    """Install the axon NTFF profiling hook if the image's antenv lacks
    axon_hooks (degrades silently; tracing is optional)."""
    try:
        from antenv.axon_hooks import get_axon_ntff_profile_hook

        if get_axon_ntff_profile_hook() is not None:
            return
        have_module = True
    except ImportError:
        have_module = False
    try:
        import antenv

        if not have_module:
            ext = "/tmp/axon_hooks_ext"
            os.makedirs(ext, exist_ok=True)
            p = os.path.join(ext, "axon_hooks.py")
            if not os.path.exists(p):
                with open(p, "w") as f:
                    f.write(_HOOK_SRC)
            if ext not in antenv.__path__:
                antenv.__path__.append(ext)
        from antenv.axon_hooks import (
            get_axon_ntff_profile_hook,
            set_axon_ntff_profile_hook,
        )

        if get_axon_ntff_profile_hook() is None:
            from trn_agent_boot.trn_boot import _ntff_profile_via_ctypes

            hook = _ntff_profile_via_ctypes("/opt/axon/libaxon_pjrt.so")
            if hook is not None:
                set_axon_ntff_profile_hook(hook)
    except Exception:
        pass


F8 = mybir.dt.float8e4
DRMODE = mybir.MatmulPerfMode.DoubleRow

# Per-step precision schedule (from numpy error study, budget 2e-2):
# L2 in fp8-DoubleRow except the 4 most error-sensitive steps (26,27,50,51
# carry ~50% of the full-fp8 variance); L3 bf16 (fp8-L3 buys <3% speed once
# the ACT/DVE elementwise wall binds, not worth the error). fp8-DR
# contracts K=256 per pass = 2x bf16 MAC rate on the PE (measured 221.6ns
# per 512-col matmul, same wall-time as a K=128 bf16 matmul).
L2_SKIP = {26, 27, 50, 51}
L3_ON = (set(range(0, 8)) | set(range(16, 24)) | set(range(40, 48))
         | set(range(56, 65)))
STEP_CFG = [(st not in L2_SKIP, st in L3_ON) for st in range(S)]


def build_program_v4(nsteps=S, nmacro=NMACRO, cfg=None):
    """v4: per-step mixed-precision. L1 always bf16 (block-diag tiles);
    L2/L3 per STEP_CFG either bf16 (v3 path) or fp8 DoubleRow (K=256 in
    one matmul). h1/h2 relu outputs are written directly in the dtype the
    next matmul needs (fp8 conversion is free in the ACT/DVE op).
    PSUM: h1a+h1b 4 banks, h2A 2 banks, h2B 1 bank, pD 1 bank; half-B's
    mc1 L2 output goes to the pD bank (sequentially reused by L3)."""
    if cfg is None:
        cfg = STEP_CFG
    half = nmacro * FD
    nc = bacc.Bacc("TRN2", target_bir_lowering=False, debug=False)

    xt = nc.dram_tensor("xt", [128, half], BF16, kind="ExternalInput")
    wb_d = nc.dram_tensor("wb", [128, nsteps * 512], BF16, kind="ExternalInput")
    w23b_d = nc.dram_tensor("w23b", [128, nsteps * 1024], BF16,
                            kind="ExternalInput")
    w23f_d = nc.dram_tensor("w23f", [128, nsteps * 1024], F8,
                            kind="ExternalInput")
    s_d = nc.dram_tensor("sv", [128, 1], F32, kind="ExternalInput")
    # bf16 output: halves the output DMA and enables the DVE 2x mode on
    # the exp(s) scale op; adds only ~0.2% per-element rounding on the
    # final values (1.604e-2 -> 1.606e-2 total, negligible vs 2e-2 gate)
    out_d = nc.dram_tensor("out", [128, half], BF16, kind="ExternalOutput")

    with TileContext(nc) as tc:
        with (
            tc.tile_pool(name="zpool", bufs=1) as zp,
            tc.tile_pool(name="consts", bufs=1) as cp,
            tc.tile_pool(name="w1pool", bufs=4) as w1p,
            tc.tile_pool(name="w23bpool", bufs=4) as w23bp,
            tc.tile_pool(name="w23fpool", bufs=4) as w23fp,
            tc.tile_pool(name="h1pool", bufs=4) as h1p,
            tc.tile_pool(name="h2pool", bufs=4) as h2p,
            tc.tile_pool(name="opool", bufs=3) as op,
            tc.tile_pool(name="psH1", bufs=1, space="PSUM") as pH1,
            tc.tile_pool(name="psH2", bufs=1, space="PSUM") as pH2,
            tc.tile_pool(name="psD", bufs=1, space="PSUM") as pD,
        ):
            # sv carries exp(s) precomputed on host (device Exp is a table
            # approximation; exact host value removes a ~1e-3 noise floor)
            exps = cp.tile([128, 1], F32, tag="exps")
            nc.sync.dma_start(exps[:], s_d[:])

            def fetch_weights(st):
                l2f8, l3f8 = cfg[st]
                wt1 = w1p.tile([128, 512], BF16, tag="w1")
                nc.sync.dma_start(wt1[:], wb_d[:, bass.ts(st, 512)])
                wtb = wtf = None
                if l2f8 or l3f8:
                    wtf = w23fp.tile([128, 8, 128], F8, tag="wf")
                if not (l2f8 and l3f8):
                    wtb = w23bp.tile([128, 1024], BF16, tag="wb")
                w0 = st * 1024
                if l2f8:
                    nc.sync.dma_start(wtf[:, 0:4, :], w23f_d[:, w0 : w0 + 512])
                else:
                    nc.sync.dma_start(wtb[:, 0:512], w23b_d[:, w0 : w0 + 512])
                if l3f8:
                    nc.sync.dma_start(wtf[:, 4:8, :],
                                      w23f_d[:, w0 + 512 : w0 + 1024])
                else:
                    nc.sync.dma_start(wtb[:, 512:1024],
                                      w23b_d[:, w0 + 512 : w0 + 1024])
                return (wt1, wtb, wtf)

            zt = zp.tile([128, half], BF16, tag="z")
            for m in range(nmacro):
                nc.gpsimd.dma_start(zt[:, bass.ts(m, FD)], xt[:, bass.ts(m, FD)])

            wts = {0: fetch_weights(0)}
            if nsteps > 1:
                wts[1] = fetch_weights(1)

            def stageA_mm(st, m):
                """L1 (bf16): 4 block-diag MMs into two 2-bank psum tiles
                (h1a = half A [k0|k1], h1b = half B) so the next macro's L1
                only waits on the matching half's evacuation."""
                wt1 = wts[st][0]
                zsl = zt[:, bass.ts(m, FD)]
                h1a = pH1.tile([128, 2, FD], F32, tag="h1a")
                h1b = pH1.tile([128, 2, FD], F32, tag="h1b")
                nc.tensor.matmul(h1a[:, 0, :], wt1[:, 0:128], zsl)
                nc.tensor.matmul(h1b[:, 0, :], wt1[:, 128:256], zsl)
                nc.tensor.matmul(h1a[:, 1, :], wt1[:, 256:384], zsl)
                nc.tensor.matmul(h1b[:, 1, :], wt1[:, 384:512], zsl)
                return h1a, h1b

            def stageA_relu(st, m, h1ps):
                """h1 evacuation: half A on ACT, half B on DVE (emitted
                after stage B so engine queue order matches dependency
                order)."""
                l2f8 = cfg[st][0]
                h1a, h1b = h1ps
                if l2f8:
                    h1wA = h1p.tile([128, 2, FD], F8, tag="h1w8A")
                    h1wB = h1p.tile([128, 2, FD], F8, tag="h1w8B")
                else:
                    h1wA = h1p.tile([128, 2 * FD], BF16, tag="h1wA")
                    h1wB = h1p.tile([128, 2 * FD], BF16, tag="h1wB")
                nc.scalar.activation(h1wA[:], h1a[:], AF.Relu)
                nc.vector.tensor_scalar(h1wB[:], h1b[:], 0.0, None, op0=ALU.max)
                return h1wA, h1wB

            def stageB(st, m, h1w):
                """L2 + h2 relus + L3 + z add, per-step precision.
                h1w layout: subtiles [A-k0, A-k1, B-k0, B-k1]."""
                l2f8, l3f8 = cfg[st]
                _, wtb, wtf = wts[st]
                zsl = zt[:, bass.ts(m, FD)]
                h1wA, h1wB = h1w
                if l2f8:
                    h1A, h1B = h1wA[:], h1wB[:]
                else:
                    h1A = (h1wA[:, 0:FD], h1wA[:, FD : 2 * FD])
                    h1B = (h1wB[:, 0:FD], h1wB[:, FD : 2 * FD])

                h2A = pH2.tile([128, 2 * FD], F32, tag="h2A")
                h2B = pH2.tile([128, FD], F32, tag="h2B")
                dps = pD.tile([128, FD], F32, tag="dps")

                if l3f8:
                    h2wA = h2p.tile([128, 2, FD], F8, tag="h2w8A")
                    h2wB = h2p.tile([128, 2, FD], F8, tag="h2w8B")
                    h2wA_f, h2wB_f = h2wA[:], h2wB[:]
                    h2wA_m = (h2wA[:, 0, :], h2wA[:, 1, :])
                    h2wB_m = (h2wB[:, 0, :], h2wB[:, 1, :])
                else:
                    h2wA = h2p.tile([128, 2 * FD], BF16, tag="h2wA")
                    h2wB = h2p.tile([128, 2 * FD], BF16, tag="h2wB")
                    h2wA_f, h2wB_f = h2wA[:], h2wB[:]
                    h2wA_m = (h2wA[:, 0:FD], h2wA[:, FD : 2 * FD])
                    h2wB_m = (h2wB[:, 0:FD], h2wB[:, FD : 2 * FD])

                # --- L2 matmuls ---
                if l2f8:
                    nc.tensor.matmul(h2A[:, 0:FD], wtf[:, 0:2, :], h1A,
                                     perf_mode=DRMODE)
                    nc.tensor.matmul(h2A[:, FD : 2 * FD], wtf[:, 2:4, :],
                                     h1A, perf_mode=DRMODE)
                    nc.tensor.matmul(h2B[:], wtf[:, 0:2, :], h1B,
                                     perf_mode=DRMODE)
                    # half-B mc1 goes to the pD bank (L3 reuses it after)
                    nc.tensor.matmul(dps[:], wtf[:, 2:4, :], h1B,
                                     perf_mode=DRMODE)
                else:
                    w2 = [wtb[:, 128 * i : 128 * (i + 1)] for i in range(4)]
                    nc.tensor.matmul(h2A[:, 0:FD], w2[0], h1A[0],
                                     start=True, stop=False)
                    nc.tensor.matmul(h2A[:, 0:FD], w2[1], h1A[1],
                                     start=False, stop=True)
                    nc.tensor.matmul(h2A[:, FD : 2 * FD], w2[2], h1A[0],
                                     start=True, stop=False)
                    nc.tensor.matmul(h2A[:, FD : 2 * FD], w2[3], h1A[1],
                                     start=False, stop=True)
                    nc.tensor.matmul(h2B[:], w2[0], h1B[0],
                                     start=True, stop=False)
                    nc.tensor.matmul(h2B[:], w2[1], h1B[1],
                                     start=False, stop=True)
                    nc.tensor.matmul(dps[:], w2[2], h1B[0],
                                     start=True, stop=False)
                    nc.tensor.matmul(dps[:], w2[3], h1B[1],
                                     start=False, stop=True)

                # --- h2 relus: ACT gets A (merged) + B-mc1, DVE gets B-mc0.
                # (Splitting the B-mc1 evac across ACT+DVE to rebalance was
                # tried and regressed 35% — two engines reading the same
                # PSUM bank serializes the pipeline.) ---
                nc.scalar.activation(h2wA_f, h2A[:], AF.Relu)
                nc.vector.tensor_scalar(h2wB_m[0], h2B[:], 0.0, None,
                                        op0=ALU.max)
                nc.scalar.activation(h2wB_m[1], dps[:], AF.Relu)

                # --- L3 matmuls (dps bank reused after B-mc1 evacuated) ---
                if l3f8:
                    nc.tensor.matmul(dps[:], wtf[:, 4:6, :], h2wA_f,
                                     start=True, stop=False, perf_mode=DRMODE)
                    nc.tensor.matmul(dps[:], wtf[:, 6:8, :], h2wB_f,
                                     start=False, stop=True, perf_mode=DRMODE)
                else:
                    w3 = [wtb[:, 512 + 128 * i : 640 + 128 * i] for i in range(4)]
                    nc.tensor.matmul(dps[:], w3[0], h2wA_m[0],
                                     start=True, stop=False)
                    nc.tensor.matmul(dps[:], w3[1], h2wA_m[1],
                                     start=False, stop=False)
                    nc.tensor.matmul(dps[:], w3[2], h2wB_m[0],
                                     start=False, stop=False)
                    nc.tensor.matmul(dps[:], w3[3], h2wB_m[1],
                                     start=False, stop=True)

                def tail(dps=dps, zsl=zsl, st=st, m=m):
                    # zadd emitted after the next macro's h1 relus so the
                    # DVE queue order (h1wB first) matches dependency order
                    nc.vector.tensor_add(zsl, dps[:], zsl)
                    if st == nsteps - 1:
                        ostage = op.tile([128, FD], BF16, tag="ostage")
                        nc.vector.tensor_scalar_mul(ostage[:], zsl, exps[:])
                        nc.sync.dma_start(out_d[:, bass.ts(m, FD)], ostage[:])
                return tail

            # Pipeline emission per macro: [L1-mms(m)] [stageB-front(m-1)]
            # [h1-relus(m)] [zadd(m-1)] so each engine's queue order matches
            # dependency readiness order. (A depth-8 step-wavefront
            # interleave of macros across steps was tested to average the
            # PE-bound/ACT-bound step types: regressed ~39us — the deeper
            # weight rotation cost more than the averaging recovered.)
            flat = [(st, m) for st in range(nsteps) for m in range(nmacro)]
            pending = None  # (st, m, h1w) awaiting stageB
            for k, (st, m) in enumerate(flat):
                if m == nmacro // 2 and st + 2 < nsteps:
                    wts[st + 2] = fetch_weights(st + 2)
                h1ps = stageA_mm(st, m)
                tail = None
                if pending is not None:
                    pst, pm, ph1w = pending
                    tail = stageB(pst, pm, ph1w)
                h1w = stageA_relu(st, m, h1ps)
                if tail is not None:
                    tail()
                pending = (st, m, h1w)
            pst, pm, ph1w = pending
            stageB(pst, pm, ph1w)()

    nc.finalize()
    return nc


def build_program_v2(nsteps=S, nmacro=NMACRO):
    """v3: all matmuls in uniform 128x128 array mode (no tiling-mode drains).

    L1 uses block-diagonal weight tiles ([W1e;0] / [0;W1e]) so each half's
    h1 comes from a full-K matmul over the shared z tile. L3 uses M=128
    zero-padded W3e tiles so both halves' updates accumulate into one PSUM
    bank. Emission is software-pipelined: stage A (L1+relu) runs one macro
    ahead of stage B (L2+relu+L3+zadd). Weights stream per step through a
    rotating pool with 2-step prefetch.
    """
    half = nmacro * FD
    nc = bacc.Bacc("TRN2", target_bir_lowering=False, debug=False)

    xt = nc.dram_tensor("xt", [128, half], BF16, kind="ExternalInput")
    wq = nc.dram_tensor("wq", [128, nsteps * WCOLS], BF16, kind="ExternalInput")
    s_d = nc.dram_tensor("sv", [128, 1], F32, kind="ExternalInput")
    out_d = nc.dram_tensor("out", [128, half], F32, kind="ExternalOutput")

    with TileContext(nc) as tc:
        with (
            tc.tile_pool(name="zpool", bufs=1) as zp,
            tc.tile_pool(name="consts", bufs=1) as cp,
            tc.tile_pool(name="wpool", bufs=4) as wp,
            tc.tile_pool(name="h1pool", bufs=4) as h1p,
            tc.tile_pool(name="h2pool", bufs=4) as h2p,
            tc.tile_pool(name="opool", bufs=3) as op,
            tc.tile_pool(name="psH1", bufs=1, space="PSUM") as pH1,
            tc.tile_pool(name="psH2", bufs=1, space="PSUM") as pH2,
            tc.tile_pool(name="psD", bufs=1, space="PSUM") as pD,
        ):
            # --- constants ---
            ss = cp.tile([128, 1], F32, tag="ss")
            nc.sync.dma_start(ss[:], s_d[:])
            exps = cp.tile([128, 1], F32, tag="exps")
            nc.scalar.activation(exps[:], ss[:], AF.Exp)

            def fetch_weights(st):
                wt = wp.tile([128, WCOLS], BF16, tag="w")
                nc.sync.dma_start(wt[:], wq[:, bass.ts(st, WCOLS)])
                return wt

            # --- z state, resident in SBUF ---
            zt = zp.tile([128, half], BF16, tag="z")
            for m in range(nmacro):
                nc.gpsimd.dma_start(zt[:, bass.ts(m, FD)], xt[:, bass.ts(m, FD)])

            wts = {0: fetch_weights(0)}
            if nsteps > 1:
                wts[1] = fetch_weights(1)

            def stageA(st, m):
                """L1 for (st, m): 4 block-diag MMs + h1 relus."""
                wt = wts[st]
                zsl = zt[:, bass.ts(m, FD)]
                h1a = pH1.tile([128, 2 * FD], F32, tag="h1a")
                h1b = pH1.tile([128, 2 * FD], F32, tag="h1b")
                # tiles: [W1e_c0;0] [0;W1e_c0] [W1e_c1;0] [0;W1e_c1]
                nc.tensor.matmul(h1a[:, 0:FD], wt[:, 0:128], zsl)
                nc.tensor.matmul(h1b[:, 0:FD], wt[:, 128:256], zsl)
                nc.tensor.matmul(h1a[:, FD : 2 * FD], wt[:, 256:384], zsl)
                nc.tensor.matmul(h1b[:, FD : 2 * FD], wt[:, 384:512], zsl)
                h1wA = h1p.tile([128, 2 * FD], BF16, tag="h1wA")
                h1wB = h1p.tile([128, 2 * FD], BF16, tag="h1wB")
                nc.scalar.activation(h1wA[:], h1a[:], AF.Relu)
                nc.vector.tensor_scalar(h1wB[:], h1b[:], 0.0, None, op0=ALU.max)
                return h1wA, h1wB

            def stageB(st, m, h1w):
                """L2 + h2 relus + L3 (M=128-padded, one bank) + z add."""
                wt = wts[st]
                h1wA, h1wB = h1w
                zsl = zt[:, bass.ts(m, FD)]
                w2 = [wt[:, 512 + 128 * i : 640 + 128 * i] for i in range(4)]
                w3 = [wt[:, 1024 + 128 * i : 1152 + 128 * i] for i in range(4)]

                h2ts = []
                h2w = {}
                for hi, (hw, hx) in enumerate(((h1wA, "A"), (h1wB, "B"))):
                    for mc in range(2):
                        if hi == 1 and mc == 1:
                            ps = h2ts[0]  # reuse A-m0 bank (PSUM budget)
                        else:
                            ps = pH2.tile([128, FD], F32, tag=f"h2_{len(h2ts)}")
                            h2ts.append(ps)
                        nc.tensor.matmul(
                            ps[:], w2[2 * mc], hw[:, 0:FD], start=True, stop=False
                        )
                        nc.tensor.matmul(
                            ps[:], w2[2 * mc + 1], hw[:, FD : 2 * FD],
                            start=False, stop=True,
                        )
                        if mc == 0:
                            h2wt = h2p.tile([128, 2 * FD], BF16, tag=f"h2w{hx}")
                            h2w[hx] = h2wt
                        osl = h2w[hx][:, mc * FD : (mc + 1) * FD]
                        if hi == 1 and mc == 1:
                            nc.vector.tensor_scalar(
                                osl, ps[:], 0.0, None, op0=ALU.max
                            )
                        else:
                            nc.scalar.activation(osl, ps[:], AF.Relu)

                dps = pD.tile([128, FD], F32, tag="dps")
                nc.tensor.matmul(
                    dps[:], w3[0], h2w["A"][:, 0:FD], start=True, stop=False
                )
                nc.tensor.matmul(
                    dps[:], w3[1], h2w["A"][:, FD : 2 * FD], start=False, stop=False
                )
                nc.tensor.matmul(
                    dps[:], w3[2], h2w["B"][:, 0:FD], start=False, stop=False
                )
                nc.tensor.matmul(
                    dps[:], w3[3], h2w["B"][:, FD : 2 * FD], start=False, stop=True
                )
                nc.vector.tensor_add(zsl, dps[:], zsl)
                if st == nsteps - 1:
                    # last step: scale + store this macro right away so the
                    # output phase overlaps the remaining macros' compute
                    ostage = op.tile([128, FD], F32, tag="ostage")
                    nc.vector.tensor_scalar_mul(ostage[:], zsl, exps[:])
                    nc.sync.dma_start(out_d[:, bass.ts(m, FD)], ostage[:])

            # --- software-pipelined main loop: A one macro ahead of B ---
            flat = [(st, m) for st in range(nsteps) for m in range(nmacro)]
            pending = None  # (st, m, h1w)
            for k, (st, m) in enumerate(flat):
                # prefetch next step's weights mid-step
                if m == nmacro // 2 and st + 2 < nsteps:
                    wts[st + 2] = fetch_weights(st + 2)
                h1w = stageA(st, m)
                if pending is not None:
                    pst, pm, ph1w = pending
                    stageB(pst, pm, ph1w)
                pending = (st, m, h1w)
            pst, pm, ph1w = pending
            stageB(pst, pm, ph1w)

    nc.finalize()
    return nc


def host_prep_v4(x, s, W1, W2, W3, idx):
    """v4 weight packing: L1 bf16 tensor + L2/L3 in both bf16 and fp8."""
    import ml_dtypes

    wq, _ = _host_pack_f32(W1, W2, W3, idx)
    sv = np.exp(np.asarray(s, np.float64)).astype(np.float32).reshape(N, 1)
    sv = np.ascontiguousarray(np.concatenate([sv, sv], axis=0))  # [128, 1]
    wb = np.empty((128, S * 512), np.float32)
    w23 = np.empty((128, S * 1024), np.float32)
    for st in range(S):
        wb[:, st * 512 : (st + 1) * 512] = wq[:, st * WCOLS : st * WCOLS + 512]
        w23[:, st * 1024 : (st + 1) * 1024] = wq[
            :, st * WCOLS + 512 : (st + 1) * WCOLS
        ]
    return (
        wb.astype(ml_dtypes.bfloat16),
        w23.astype(ml_dtypes.bfloat16),
        w23.astype(ml_dtypes.float8_e4m3),
        sv,
    )


def _host_pack_f32(W1, W2, W3, idx):
    """Shared f32 packing (v3 wq layout)."""
    W1 = np.asarray(W1, np.float32)
    W2 = np.asarray(W2, np.float32)
    W3 = np.asarray(W3, np.float32)
    idx = np.asarray(idx)

    wq = np.zeros((128, S * WCOLS), np.float32)
    for st in range(S):
        i = int(idx[st])
        W1e = np.zeros((N, H), np.float32)
        W1e[: N - 1] = W1[st]
        W1e[[i, N - 1]] = W1e[[N - 1, i]]
        W3e = np.zeros((H, N), np.float32)
        W3e[:, i] = W3[st, :, 0]
        w0 = st * WCOLS
        wq[0:64, w0 + 0 : w0 + 128] = W1e[:, 0:128]
        wq[64:128, w0 + 128 : w0 + 256] = W1e[:, 0:128]
        wq[0:64, w0 + 256 : w0 + 384] = W1e[:, 128:256]
        wq[64:128, w0 + 384 : w0 + 512] = W1e[:, 128:256]
        wq[:, w0 + 512 : w0 + 640] = W2[st, 0:128, 0:128]
        wq[:, w0 + 640 : w0 + 768] = W2[st, 128:256, 0:128]
        wq[:, w0 + 768 : w0 + 896] = W2[st, 0:128, 128:256]
        wq[:, w0 + 896 : w0 + 1024] = W2[st, 128:256, 128:256]
        wq[:, w0 + 1024 : w0 + 1088] = W3e[0:128, :]
        wq[:, w0 + 1152 : w0 + 1216] = W3e[128:256, :]
        wq[:, w0 + 1344 : w0 + 1408] = W3e[0:128, :]
        wq[:, w0 + 1472 : w0 + 1536] = W3e[128:256, :]
    sv = None
    return wq, sv


def host_prep_v2(x, s, W1, W2, W3, idx):
    """Build per-step packed weights and the split-sample transposed x."""
    import ml_dtypes

    W1 = np.asarray(W1, np.float32)
    W2 = np.asarray(W2, np.float32)
    W3 = np.asarray(W3, np.float32)
    idx = np.asarray(idx)

    wq = np.zeros((128, S * WCOLS), np.float32)
    for st in range(S):
        i = int(idx[st])
        W1e = np.zeros((N, H), np.float32)
        W1e[: N - 1] = W1[st]
        W1e[[i, N - 1]] = W1e[[N - 1, i]]
        W3e = np.zeros((H, N), np.float32)
        W3e[:, i] = W3[st, :, 0]
        w0 = st * WCOLS
        # L1 block-diagonal tiles: [W1e_c;0] for half A, [0;W1e_c] for B
        wq[0:64, w0 + 0 : w0 + 128] = W1e[:, 0:128]
        wq[64:128, w0 + 128 : w0 + 256] = W1e[:, 0:128]
        wq[0:64, w0 + 256 : w0 + 384] = W1e[:, 128:256]
        wq[64:128, w0 + 384 : w0 + 512] = W1e[:, 128:256]
        # W2 tiles (k-chunk-major within each m-chunk)
        wq[:, w0 + 512 : w0 + 640] = W2[st, 0:128, 0:128]
        wq[:, w0 + 640 : w0 + 768] = W2[st, 128:256, 0:128]
        wq[:, w0 + 768 : w0 + 896] = W2[st, 0:128, 128:256]
        wq[:, w0 + 896 : w0 + 1024] = W2[st, 128:256, 128:256]
        # W3e M=128-padded tiles: cols 0:64 update half A, 64:128 half B
        wq[:, w0 + 1024 : w0 + 1088] = W3e[0:128, :]
        wq[:, w0 + 1152 : w0 + 1216] = W3e[128:256, :]
        wq[:, w0 + 1344 : w0 + 1408] = W3e[0:128, :]
        wq[:, w0 + 1472 : w0 + 1536] = W3e[128:256, :]
    wq = wq.astype(ml_dtypes.bfloat16)

    sv = np.asarray(s, np.float32).reshape(N, 1)
    sv = np.concatenate([sv, sv], axis=0)  # [128, 1]
    return wq, sv


_PROGRAM_V2 = None
_RUN_IDX = 0


def kernel(x, s, W1, b1, W2, b2, W3, b3, idx):
    global LAST_RESULT, _PROGRAM_V2
    use_bias = bool(
        np.abs(b1).max() > 0 or np.abs(b2).max() > 0 or np.abs(b3).max() > 0
    )
    if use_bias:
        return _kernel_v1(x, s, W1, b1, W2, b2, W3, b3, idx)

    x = np.asarray(x, np.float32)
    wb, w23b, w23f, sv = host_prep_v4(x, s, W1, W2, W3, idx)
    in_maps = []
    for c in range(NCORES):
        xc = x[c * BSH : (c + 1) * BSH]
        xts = np.empty((128, HALF), np.float32)
        xts[0:64] = xc[0:HALF].T
        xts[64:128] = xc[HALF:BSH].T
        import ml_dtypes
        in_maps.append(
            dict(xt=np.ascontiguousarray(xts).astype(ml_dtypes.bfloat16),
                 wb=wb, w23b=w23b, w23f=w23f, sv=sv)
        )

    if _PROGRAM_V2 is None:
        _PROGRAM_V2 = build_program_v4()
    _ensure_ntff_hook()
    global _RUN_IDX
    tmpdir = os.environ.get("KERNEL_TMPDIR")
    if tmpdir:
        tmpdir = os.path.join(tmpdir, f"run{_RUN_IDX}")
        _RUN_IDX += 1
        os.makedirs(tmpdir, exist_ok=True)
    res = run_bass_kernel_spmd(
        _PROGRAM_V2, in_maps, core_ids=list(range(NCORES)), tmpdir=tmpdir
    )
    LAST_RESULT = res
    out = np.empty((B, N), np.float32)
    for c in range(NCORES):
        o = np.asarray(res.results[c]["out"]).astype(np.float32)  # [128, HALF]
        out[c * BSH : c * BSH + HALF] = o[0:64].T
        out[c * BSH + HALF : (c + 1) * BSH] = o[64:128].T
    return out


# ---------------------------------------------------------------------------
# v1 fallback (baseline) — used only when biases are nonzero.
# ---------------------------------------------------------------------------
TILE = 512
MACRO = 1024
_PROGRAM_V1 = {}


def build_program_v1(nsteps=S, nmacro=BSH // MACRO, use_bias=True, hbufs=3):
    bsh = nmacro * MACRO
    nc = bacc.Bacc("TRN2", target_bir_lowering=False, debug=False)

    xt = nc.dram_tensor("xt", [N, bsh], BF16, kind="ExternalInput")
    wp_d = nc.dram_tensor("wpack", [nsteps, 128, 896], BF16, kind="ExternalInput")
    b1_d = nc.dram_tensor("b1r", [128, 2 * nsteps], F32, kind="ExternalInput")
    b2_d = nc.dram_tensor("b2r", [128, 2 * nsteps], F32, kind="ExternalInput")
    b3_d = nc.dram_tensor("b3c", [N, nsteps], F32, kind="ExternalInput")
    s_d = nc.dram_tensor("sv", [N, 1], F32, kind="ExternalInput")
    out_d = nc.dram_tensor("out", [N, bsh], F32, kind="ExternalOutput")

    with TileContext(nc) as tc:
        with (
            tc.tile_pool(name="zpool", bufs=1) as zp,
            tc.tile_pool(name="consts", bufs=1) as cp,
            tc.tile_pool(name="wpool", bufs=4) as wp,
            tc.tile_pool(name="hpool", bufs=hbufs) as hp,
            tc.tile_pool(name="psA", bufs=3, space="PSUM") as pA,
            tc.tile_pool(name="psB", bufs=3, space="PSUM") as pB,
            tc.tile_pool(name="psZ", bufs=2, space="PSUM") as pZ,
        ):
            if use_bias:
                b1s = cp.tile([128, 2 * nsteps], F32, tag="b1s")
                nc.sync.dma_start(b1s[:], b1_d[:])
                b2s = cp.tile([128, 2 * nsteps], F32, tag="b2s")
                nc.sync.dma_start(b2s[:], b2_d[:])
                b3s = cp.tile([N, nsteps], F32, tag="b3s")
                nc.sync.dma_start(b3s[:], b3_d[:])
            ss = cp.tile([N, 1], F32, tag="ss")
            nc.sync.dma_start(ss[:], s_d[:])
            exps = cp.tile([N, 1], F32, tag="exps")
            nc.scalar.activation(exps[:], ss[:], AF.Exp)

            def fetch_weights(st):
                wt = wp.tile([128, 896], BF16, tag="w")
                nc.sync.dma_start(wt[:], wp_d[st])
                return (
                    wt[0:N, 0:H], wt[:, 256:512], wt[:, 512:768],
                    wt[:, 768:832], wt[:, 832:896],
                )

            wtiles = fetch_weights(0)
            zt = zp.tile([N, bsh], BF16, tag="z")
            for m in range(nmacro):
                msl = bass.ts(m, MACRO)
                nc.gpsimd.dma_start(zt[:, msl], xt[:, msl])

            pending_l3 = None
            for st in range(nsteps):
                if st > 0:
                    wtiles = fetch_weights(st)
                w1t, w2ta, w2tb, w3ta, w3tb = wtiles

                for m in range(nmacro):
                    zsl = zt[:, bass.ts(m, MACRO)]

                    def act_relu(out, in_, bcol):
                        if use_bias:
                            nc.scalar.activation(out, in_, AF.Relu, bias=bcol)
                        else:
                            nc.scalar.activation(out, in_, AF.Relu)

                    def dve_relu(out, in_, bcol):
                        if use_bias:
                            nc.vector.tensor_scalar(
                                out, in_, bcol, 0.0, op0=ALU.add, op1=ALU.max
                            )
                        else:
                            nc.vector.tensor_scalar(out, in_, 0.0, None, op0=ALU.max)

                    b1a = b1s[:, 2 * st : 2 * st + 1] if use_bias else None
                    b1b = b1s[:, 2 * st + 1 : 2 * st + 2] if use_bias else None
                    b2a = b2s[:, 2 * st : 2 * st + 1] if use_bias else None
                    b2b = b2s[:, 2 * st + 1 : 2 * st + 2] if use_bias else None

                    h1ps = []
                    for t in range(MACRO // TILE):
                        tsl = bass.ts(t, TILE)
                        pa = pA.tile([128, TILE], F32, tag="h1p")
                        pb = pA.tile([128, TILE], F32, tag="h1p")
                        nc.tensor.matmul(pa[:], w1t[:, 0:128], zsl[:, tsl])
                        nc.tensor.matmul(pb[:], w1t[:, 128:256], zsl[:, tsl])
                        h1ps.append((pa, pb))
                    if pending_l3 is not None:
                        pending_l3()
                        pending_l3 = None
                    h1a = hp.tile([128, MACRO], BF16, tag="h1a")
                    h1b = hp.tile([128, MACRO], BF16, tag="h1b")
                    act_relu(h1a[:, 0:TILE], h1ps[0][0][:], b1a)
                    dve_relu(h1b[:, 0:TILE], h1ps[0][1][:], b1b)
                    act_relu(h1a[:, TILE:MACRO], h1ps[1][0][:], b1a)
                    act_relu(h1b[:, TILE:MACRO], h1ps[1][1][:], b1b)

                    h2a = hp.tile([128, MACRO], BF16, tag="h2a")
                    h2b = hp.tile([128, MACRO], BF16, tag="h2b")
                    for t in range(MACRO // TILE):
                        tsl = bass.ts(t, TILE)
                        pa = pB.tile([128, TILE], F32, tag="h2p")
                        pb = pB.tile([128, TILE], F32, tag="h2p")
                        nc.tensor.matmul(
                            pa[:], w2ta[:, 0:128], h1a[:, tsl], start=True, stop=False
                        )
                        nc.tensor.matmul(
                            pa[:], w2tb[:, 0:128], h1b[:, tsl], start=False, stop=True
                        )
                        nc.tensor.matmul(
                            pb[:], w2ta[:, 128:256], h1a[:, tsl], start=True, stop=False
                        )
                        nc.tensor.matmul(
                            pb[:], w2tb[:, 128:256], h1b[:, tsl], start=False, stop=True
                        )
                        act_relu(h2a[:, tsl], pa[:], b2a)
                        dve_relu(h2b[:, tsl], pb[:], b2b)

                    def emit_l3(h2a=h2a, h2b=h2b, zsl=zsl, w3ta=w3ta, w3tb=w3tb, st=st):
                        for t in range(MACRO // TILE):
                            tsl = bass.ts(t, TILE)
                            zps = pZ.tile([N, TILE], F32, tag="zp")
                            nc.tensor.matmul(
                                zps[:], w3ta[:], h2a[:, tsl], start=True, stop=False
                            )
                            nc.tensor.matmul(
                                zps[:], w3tb[:], h2b[:, tsl], start=False, stop=True
                            )
                            ztile = zsl[:, tsl]
                            if use_bias:
                                nc.vector.scalar_tensor_tensor(
                                    ztile, zps[:], b3s[:, st : st + 1], ztile,
                                    op0=ALU.add, op1=ALU.add,
                                )
                            else:
                                nc.vector.tensor_add(ztile, zps[:], ztile)

                    pending_l3 = emit_l3

            if pending_l3 is not None:
                pending_l3()

            for m in range(nmacro):
                msl = bass.ts(m, MACRO)
                ostage = hp.tile([N, MACRO], F32, tag="ostage")
                nc.vector.tensor_scalar_mul(ostage[:], zt[:, msl], exps[:])
                nc.sync.dma_start(out_d[:, msl], ostage[:])

    nc.finalize()
    return nc


def _host_prep_v1(x, s, W1, b1, W2, b2, W3, b3, idx, nsteps=S):
    x = np.asarray(x, np.float32)
    idx = np.asarray(idx)
    W1 = np.asarray(W1, np.float32)
    W2 = np.ascontiguousarray(np.asarray(W2, np.float32)[:nsteps])
    W3 = np.asarray(W3, np.float32)
    b1 = np.asarray(b1, np.float32)
    b2 = np.asarray(b2, np.float32)
    b3 = np.asarray(b3, np.float32)

    W1e = np.zeros((nsteps, N, H), np.float32)
    W1e[:, : N - 1, :] = W1[:nsteps]
    for st in range(nsteps):
        i = int(idx[st])
        r = W1e[st].copy()
        r[[i, N - 1]] = r[[N - 1, i]]
        W1e[st] = r
    W3e = np.zeros((nsteps, H, N), np.float32)
    for st in range(nsteps):
        W3e[st, :, int(idx[st])] = W3[st, :, 0]
    b3c = np.zeros((N, nsteps), np.float32)
    for st in range(nsteps):
        b3c[int(idx[st]), st] = b3[st, 0]
    import ml_dtypes
    wpack = np.zeros((nsteps, 128, 896), np.float32)
    wpack[:, 0:N, 0:H] = W1e
    wpack[:, :, 256:512] = W2[:, 0:128, :]
    wpack[:, :, 512:768] = W2[:, 128:256, :]
    wpack[:, :, 768:832] = W3e[:, 0:128, :]
    wpack[:, :, 832:896] = W3e[:, 128:256, :]
    b1r = np.ascontiguousarray(
        b1[:nsteps].reshape(nsteps, 2, 128).transpose(2, 0, 1).reshape(128, 2 * nsteps)
    )
    b2r = np.ascontiguousarray(
        b2[:nsteps].reshape(nsteps, 2, 128).transpose(2, 0, 1).reshape(128, 2 * nsteps)
    )
    wpack = wpack.astype(ml_dtypes.bfloat16)
    xt = np.ascontiguousarray(x.T).astype(ml_dtypes.bfloat16)
    sv = np.ascontiguousarray(np.asarray(s, np.float32).reshape(N, 1))
    return dict(wpack=wpack, b1r=b1r, b2r=b2r, b3c=b3c, sv=sv), xt


def _kernel_v1(x, s, W1, b1, W2, b2, W3, b3, idx):
    global LAST_RESULT
    shared, xt = _host_prep_v1(x, s, W1, b1, W2, b2, W3, b3, idx)
    in_maps = []
    for c in range(NCORES):
        m = dict(shared)
        m["xt"] = np.ascontiguousarray(xt[:, c * BSH : (c + 1) * BSH])
        in_maps.append(m)
    if True not in _PROGRAM_V1:
        _PROGRAM_V1[True] = build_program_v1(use_bias=True)
    _ensure_ntff_hook()
    res = run_bass_kernel_spmd(
        _PROGRAM_V1[True], in_maps, core_ids=list(range(NCORES))
    )
    LAST_RESULT = res
    outs = [res.results[c]["out"] for c in range(NCORES)]
    return np.ascontiguousarray(
        np.concatenate([o.T for o in outs], axis=0), dtype=np.float32
    )

